# revision 69
# baseline (speedup 1.0000x reference)
"""Trainium2 Bass kernel for DeterministicLSTMSensorBasedForwardDynamics.

Problem: B=4096, T=50, OBS=64, ACT=16, H=256, OUT=64, 5-layer MLP head.
  x = concat(traj, act)                     [B, T, 80]
  LSTM over T with silu cell activation (g = silu(zg), h = o * silu(c))
  MLP: 5 x (Dense(256) + silu), Dense(64)

Strategy (data parallel over 8 cores, 512 batch each):
  * Transposed layout: activations are [feature, batch]; weights are the
    PE-stationary operand, batch streams as the moving dimension.
  * The 512-batch is split into two independent 256-batch recurrence
    "braids" (A, B).  Each braid's serial chain (sigmoids -> gate arith ->
    sig(c) -> h -> Wh matmul -> next sigmoids) hides inside the other
    braid's ACT work, so the kernel is ACT-throughput-bound (~5.4us/step,
    ~98% ACT occupancy), not latency-bound.
  * h and Wh are fp8e4m3; the h-matmul per z-feature-tile is ONE DoubleRow
    matmul (K=256 as 2 k-groups) costing out_free/2 PE cycles.  The input
    path (x, Wi) stays bf16: fp8 there fails accuracy, fp8 on the
    recurrence path costs <0.1% end-to-end error.
  * All LSTM activations use ONE table set (sigmoid): gates are sigmoids
    directly and silu is reconstructed on DVE/Pool:
      g  = sig_g * zg                (DVE, reads PSUM)   = silu(zg)
      u  = sig_f * c_prev            (Pool)
      tt = sig_i * g                 (DVE, 2x bf16)
      c  = u + tt                    (Pool)
      v  = sig_o * c                 (Pool, off-path)
      sc = sigmoid(c)                (ACT)
      h  = v * sc                    (DVE, fp8 out)      = o * silu(c)
    Per braid-step ACT does just 3 ops: sigmoid(g,f,i regions) [128,1536],
    sigmoid(o) [128,512], sigmoid(c) [128,512]; g/u/tt/c all depend only
    on the first, so each braid's chain runs under the other's ACT ops.
  * PSUM: per braid one [128, 8*256] tile in gate order (g f i o), 2KB
    bank = one gate.  Accumulation groups are PER BANK (zero regions are
    whole banks): the first Wi matmul of a bank starts the group, the
    second Wh DoubleRow matmul stops it.
  * bh is folded into the Wi matmul via an all-ones input row (K=81);
    MLP biases via ones-row outer products on the PE.
  * Host prep builds x^T as bf16 [128, 4*50*128]: partitions = padded input
    feature, free = (btile, t, b); only rows 0:81 are transferred.
    MLP runs in fp32r with one silu table load at the LSTM/MLP boundary.
  * GPSIMD cannot touch PSUM and only runs TensorTensor-class ops; all
    PSUM-reading elementwise work sits on DVE.
"""

import os
import sys

sys.path.insert(0, "/opt/trn_rl_repo")

import numpy as np
import ml_dtypes

import concourse.bacc as bacc
import concourse.tile as tile
from concourse import mybir
from concourse import bass_utils

N_CORES = 8
B, T, OBS, ACTD, H, OUT, NL = 4096, 50, 64, 16, 256, 64, 5
BC = B // N_CORES          # batch per core = 512
NBT = BC // 128            # b-tiles per core = 4
HB = 256                   # braid half-batch

BF16 = mybir.dt.bfloat16
F32 = mybir.dt.float32
F32R = mybir.dt.float32r
FP8 = mybir.dt.float8e4
AF = mybir.ActivationFunctionType
ALU = mybir.AluOpType
DR = mybir.MatmulPerfMode.DoubleRow

# z-feature regions per braid PSUM tile, permuted gate order (g f i o):
# tanh1 covers (g, f) so G' and u prefetch during tanh2's (i, o)
SL_G = slice(0 * HB, 2 * HB)     # zg: regions 0,1
SL_F = slice(2 * HB, 4 * HB)     # zf: regions 2,3
SL_I = slice(4 * HB, 6 * HB)     # zi: regions 4,5
SL_O = slice(6 * HB, 8 * HB)     # zo: regions 6,7

# column permutation: reference z order (i f g o) -> (g f i o)
_PERM = np.concatenate([np.arange(512, 768), np.arange(256, 512),
                        np.arange(0, 256), np.arange(768, 1024)])

_CACHE = {}

# manual ACT cadence pinning (ns); CAD_P=0 disables
CAD_P = float(os.environ.get("CAD_P", "0"))
CAD_W0 = float(os.environ.get("CAD_W0", "8600"))
_CAD_OFF = {"t1A": 0.0, "t2A": 1038.0, "t1B": 2076.0, "t2B": 3114.0,
            "siA": 4152.0, "siB": 4764.0}


def _cad(tc, key, t):
    """Context pinning an ACT op to its cadence slot (no-op if disabled)."""
    if CAD_P <= 0 or t < 1:
        return tc.tile_wait_until(0, enable=False)
    ns = CAD_W0 + (t - 1) * CAD_P + _CAD_OFF[key]
    return tc.tile_wait_until(ns / 1e6)


def _build(t_steps=T):
    """Build + compile the Bass module (cached)."""
    if t_steps in _CACHE:
        return _CACHE[t_steps]

    nc = bacc.Bacc("TRN2", target_bir_lowering=False, debug=False,
                   num_devices=N_CORES)

    xt_d = nc.dram_tensor("xt", [128, NBT * t_steps * 128], BF16,
                          kind="ExternalInput").ap()
    wh_d = nc.dram_tensor("wh", [128, 2048], FP8, kind="ExternalInput").ap()
    wi_d = nc.dram_tensor("wi", [128, 1024], BF16, kind="ExternalInput").ap()
    mlpw_d = nc.dram_tensor("mlpw", [128, NL * 2 * 256], F32R,
                            kind="ExternalInput").ap()
    mlpb_d = nc.dram_tensor("mlpb", [1, NL * 256 + 256], F32R,
                            kind="ExternalInput").ap()
    wout_d = nc.dram_tensor("wout", [128, 128], F32R, kind="ExternalInput").ap()
    boutb_d = nc.dram_tensor("boutb", [128, 256], F32,
                             kind="ExternalInput").ap()
    pred_d = nc.dram_tensor("pred", [BC, OUT], F32, kind="ExternalOutput").ap()

    with tile.TileContext(nc) as tc:
        with (
            tc.tile_pool(name="singles", bufs=1) as singles,
            tc.tile_pool(name="hpool", bufs=3) as hpool,
            tc.tile_pool(name="work", bufs=3) as work,
            tc.tile_pool(name="psum", bufs=1, space="PSUM") as psum,
        ):
            # ---- weights / persistent state ----
            # warm the ACT table set holding BOTH tanh and silu during the
            # DMA fill, so no mid-pipeline table reload occurs
            warm = singles.tile([128, 1], BF16, tag="warm")
            ca1 = nc.const_aps.aps[(F32, 1.0)]
            nc.scalar.activation(warm[:], ca1, AF.Sigmoid)

            # spread the startup-critical transfers (wi + the first small
            # time-block of x^T per b-tile) across idle engines' DMA queues
            wi = singles.tile([128, 1024], BF16, tag="wi")
            nc.sync.dma_start(wi[:], wi_d[:])
            wh = singles.tile([128, 2048], FP8, tag="wh")
            nc.scalar.dma_start(wh[:], wh_d[:])

            xt = singles.tile([128, NBT * t_steps * 128], BF16, tag="xt")
            if t_steps >= 10:
                blks = [0, 2, 10] + list(range(20, t_steps + 1, 10))
            else:
                blks = [0, t_steps]
            first_q = [nc.gpsimd, nc.gpsimd, nc.scalar, nc.scalar]
            for t0, t1 in zip(blks[:-1], blks[1:]):
                for bt in range(NBT):
                    lo = bt * t_steps * 128 + t0 * 128
                    hi = bt * t_steps * 128 + t1 * 128
                    eng = first_q[bt] if t0 == 0 else nc.sync
                    eng.dma_start(xt[0:81, lo:hi], xt_d[0:81, lo:hi])

            # MLP-only weights are not needed until after step T-1
            mlpw = singles.tile([128, NL * 2 * 256], F32R, tag="mlpw")
            nc.sync.dma_start(mlpw[:], mlpw_d[:])
            # mlpb carries the biases + a ones-row (for bias outer products)
            mlpb = singles.tile([1, NL * 256 + 256], F32R, tag="mlpb")
            nc.sync.dma_start(mlpb[:], mlpb_d[:])
            wout = singles.tile([128, 128], F32R, tag="wout")
            nc.sync.dma_start(wout[:], wout_d[:])
            boutb = singles.tile([128, 256], F32, tag="boutb")
            nc.sync.dma_start(boutb[:], boutb_d[:])
            xt_r = xt[:].rearrange("p (bt t b) -> p bt t b", bt=NBT, t=t_steps)
            wh_r = wh[:].rearrange("p (k j) -> p k j", k=2)

            # per-braid ct = 4c state (written at t=0, no memset needed)
            ct = {X: singles.tile([128, 512], F32, tag=f"ct{X}",
                                  name=f"ct{X}") for X in "AB"}
            # h'_final for the MLP: [128, (ktile 2) * 512b]
            hlast = singles.tile([128, 1024], F32R, tag="hlast")
            hlast_r = hlast[:].rearrange("p (k b) -> p k b", k=2)

            bt0 = {"A": 0, "B": 2}
            zp = {}      # live PSUM tile per braid
            tau = {}
            G = {}
            sc = {}
            hprev = {}

            def wi_mm(X, t):
                """open zp[X]'s accumulation groups for step t.  PSUM
                zero-regions are full 2KB banks: ONE group per bank (= one
                gate = two 256-col feature regions), started by the first
                matmul touching the bank and stopped by the last."""
                zp[X] = psum.tile([128, 2048], F32, tag=f"z{X}",
                                  name=f"z{X}")
                rhs_x = xt_r[0:81, bt0[X]:bt0[X] + 2, t:t + 1, :]
                for m in [0, 1, 2, 3, 4, 5, 6, 7]:
                    nc.tensor.matmul(
                        zp[X][:, m * HB:(m + 1) * HB],
                        wi[0:81, m * 128:(m + 1) * 128], rhs_x,
                        start=(m % 2 == 0), stop=(t == 0 and m % 2 == 1))

            def wh_mm(X):
                """close the bank groups: one fp8 DoubleRow matmul per
                feature region, stop on each bank's second region"""
                h_r = hprev[X][:].rearrange("p (k b) -> p k b", k=2)
                for m in [0, 1, 2, 3, 4, 5, 6, 7]:
                    nc.tensor.matmul(
                        zp[X][:, m * HB:(m + 1) * HB],
                        wh_r[:, :, m * 128:(m + 1) * 128], h_r,
                        start=False, stop=(m % 2 == 1), perf_mode=DR)

            def sig1(X, t):
                """sigmoid of the (g, f, i) regions"""
                tau[X] = work.tile([128, 2048], BF16, tag=f"tau{X}",
                                   name=f"tau{X}")
                with _cad(tc, f"t1{X}", t):
                    nc.scalar.activation(tau[X][:, 0:1536],
                                         zp[X][:, 0:1536], AF.Sigmoid)

            def sig2(X, t):
                """sigmoid of the (o) regions"""
                with _cad(tc, f"t2{X}", t):
                    nc.scalar.activation(tau[X][:, 1536:2048],
                                         zp[X][:, 1536:2048], AF.Sigmoid)

            U = {}
            V = {}

            def g_pre(X, t):
                """g = sig_g * zg (DVE: GPSIMD cannot read PSUM), then
                u = sig_f * c (Pool) and tt = sig_i * g (DVE, 2x bf16);
                all depend only on sig1, so they run during other ACT ops"""
                G[X] = work.tile([128, 512], BF16, tag=f"G{X}",
                                 name=f"G{X}")
                nc.vector.tensor_tensor(
                    G[X][:], tau[X][:, SL_G], zp[X][:, SL_G], ALU.mult)
                if t > 0:
                    U[X] = work.tile([128, 512], F32, tag=f"u{X}",
                                     name=f"u{X}")
                    nc.gpsimd.tensor_tensor(
                        U[X][:], tau[X][:, SL_F], ct[X][:], ALU.mult)

            def gate_arith(X, t):
                """tt (DVE, right after g) then c = u + tt (Pool)"""
                if t > 0:
                    tt = work.tile([128, 512], BF16, tag=f"tt{X}",
                                   name=f"tt{X}")
                    nc.vector.tensor_tensor(
                        tt[:], tau[X][:, SL_I], G[X][:], ALU.mult)
                    nc.gpsimd.tensor_tensor(
                        ct[X][:], U[X][:], tt[:], ALU.add)
                else:
                    nc.gpsimd.tensor_tensor(
                        ct[X][:], tau[X][:, SL_I], G[X][:], ALU.mult)

            def v_pre(X, t):
                """v = sig_o * c (Pool), off the sigmoid(c) critical path"""
                V[X] = work.tile([128, 512], BF16, tag=f"v{X}",
                                 name=f"v{X}")
                nc.gpsimd.tensor_tensor(
                    V[X][:], tau[X][:, SL_O], ct[X][:], ALU.mult)

            def sig_c(X, t):
                """sigmoid(c); h = v * sig(c) = o * silu(c) follows"""
                sc[X] = work.tile([128, 512], BF16, tag=f"sc{X}",
                                  name=f"sc{X}")
                with _cad(tc, f"si{X}", t):
                    nc.scalar.activation(sc[X][:], ct[X][:], AF.Sigmoid)

            def h_out(X, t):
                last = t == t_steps - 1
                if not last:
                    # DVE: Pool is busy with the other braid's c here
                    hprev[X] = hpool.tile([128, 512], FP8, tag=f"h{X}",
                                          name=f"h{X}")
                    nc.vector.tensor_tensor(
                        hprev[X][:], V[X][:], sc[X][:], ALU.mult)
                else:
                    # f32r path for the MLP head
                    b0 = bt0[X] * 128
                    nc.vector.tensor_tensor(
                        hlast_r[:, :, b0:b0 + HB],
                        V[X][:].rearrange("p (k b) -> p k b", k=2),
                        sc[X][:].rearrange("p (k b) -> p k b", k=2),
                        ALU.mult)

            # ---- pipeline ----
            wi_mm("A", 0)
            wi_mm("B", 0)
            for t in range(t_steps):
                sig1("A", t)
                g_pre("A", t)
                gate_arith("A", t)
                sig1("B", t)
                g_pre("B", t)
                sig2("A", t)
                v_pre("A", t)
                sig_c("A", t)
                gate_arith("B", t)
                sig2("B", t)
                h_out("A", t)
                if t + 1 < t_steps:
                    wi_mm("A", t + 1)
                    wh_mm("A")
                v_pre("B", t)
                sig_c("B", t)
                h_out("B", t)
                if t + 1 < t_steps:
                    wi_mm("B", t + 1)
                    wh_mm("B")

            # ---- MLP head (fp32r), two batch halves pipelined so one
            # half's silu overlaps the other half's matmuls ----
            cur = hlast
            for layer in range(NL):
                nxt = work.tile([128, 1024], F32R, tag=f"mlp_out{layer % 2}")
                nxt_r = nxt[:].rearrange("p (m b) -> p m b", m=2)
                for hb in range(2):
                    b0 = hb * 256
                    # mp is ONE 2KB bank: a single accumulation group for
                    # both m-halves (bias outer products open it)
                    mp = psum.tile([128, 512], F32, tag="zA" if hb == 0
                                   else "zB", name=f"mlp_ps{layer}_{hb}")
                    for m in range(2):
                        nc.tensor.matmul(
                            mp[:, m * 256:(m + 1) * 256],
                            mlpb[:, layer * 256 + m * 128:
                                 layer * 256 + (m + 1) * 128],
                            mlpb[:, NL * 256:NL * 256 + 256],
                            start=(m == 0), stop=False)
                    for m in range(2):
                        for k in range(2):
                            nc.tensor.matmul(
                                mp[:, m * 256:(m + 1) * 256],
                                mlpw[:, (layer * 2 + k) * 256 + m * 128:
                                     (layer * 2 + k) * 256 + (m + 1) * 128
                                     ],
                                cur[:, k * 512 + b0:k * 512 + b0 + 256],
                                start=False, stop=(m == 1 and k == 1))
                    nc.scalar.activation(
                        nxt_r[:, :, b0:b0 + 256], mp[:], AF.Silu)
                cur = nxt

            # output layer back to [batch, OUT] layout, split per batch
            # half so the first half's store overlaps the second's matmuls:
            # lhsT = activations (stationary), rhs = Wout (moving)
            pred_r = pred_d.rearrange("(m p) f -> p m f", p=128)
            preds = singles.tile([128, 256], F32, tag="preds")
            for hb in range(2):
                pp = psum.tile([128, 128], F32, tag="zA" if hb == 0
                               else "zB", name=f"pred_ps{hb}")
                for m in range(2):
                    for k in range(2):
                        mm = hb * 2 + m
                        nc.tensor.matmul(
                            pp[:, m * 64:(m + 1) * 64],
                            cur[:, k * 512 + mm * 128:
                                k * 512 + (mm + 1) * 128],
                            wout[:, k * 64:(k + 1) * 64],
                            start=(m == 0 and k == 0),
                            stop=(m == 1 and k == 1))
                sl = slice(hb * 128, (hb + 1) * 128)
                nc.vector.tensor_add(preds[:, sl], pp[:], boutb[:, sl])
                nc.sync.dma_start(
                    pred_r[:, hb * 2:(hb + 1) * 2, :],
                    preds[:, sl].rearrange("p (m f) -> p m f", f=OUT))

    nc.compile()
    _CACHE[t_steps] = nc
    return nc


def _prep_inputs(trajectory, actions, Wi, Wh, bh, mlp_W, mlp_b, Wout, bout,
                 t_steps=T):
    """Host-side layout prep. Returns per-core input maps."""
    f32 = np.float32
    trajectory = np.asarray(trajectory, f32)
    actions = np.asarray(actions, f32)
    Wi = np.asarray(Wi, f32)
    Wh = np.asarray(Wh, f32)
    bh = np.asarray(bh, f32)
    mlp_W = np.asarray(mlp_W, f32)
    mlp_b = np.asarray(mlp_b, f32)
    Wout = np.asarray(Wout, f32)
    bout = np.asarray(bout, f32)

    # gate permutation; Wh in fp8e4m3 for DoubleRow (h is exact-scale)
    Wh_p = Wh[:, _PERM].astype(ml_dtypes.float8_e4m3)
    wh_l = Wh_p.reshape(2, 128, 1024).transpose(1, 0, 2).reshape(128, 2048)
    Wi_p = Wi[:, _PERM]
    bh_p = bh[_PERM]
    wi_l = np.zeros((128, 1024), ml_dtypes.bfloat16)
    wi_l[0:OBS] = Wi_p[0:OBS].astype(ml_dtypes.bfloat16)
    wi_l[OBS:OBS + ACTD] = Wi_p[OBS:OBS + ACTD].astype(ml_dtypes.bfloat16)
    wi_l[80] = bh_p.astype(ml_dtypes.bfloat16)

    mlpw_l = mlp_W.reshape(NL, 2, 128, 256).transpose(2, 0, 1, 3).reshape(
        128, NL * 2 * 256)
    mlpb_l = np.concatenate([mlp_b.reshape(1, NL * 256),
                             np.ones((1, 256), f32)], axis=1)
    wout_l = Wout.reshape(2, 128, 64).transpose(1, 0, 2).reshape(128, 128)
    boutb_l = np.tile(bout, (128, 4))

    in_maps = []
    for c in range(N_CORES):
        tr = trajectory[c * BC:(c + 1) * BC, :t_steps]    # [512, t, 64]
        ac = actions[c * BC:(c + 1) * BC, :t_steps]       # [512, t, 16]
        xt = np.zeros((128, NBT, t_steps, 128), ml_dtypes.bfloat16)
        xt[0:OBS] = tr.reshape(NBT, 128, t_steps, OBS).transpose(
            3, 0, 2, 1).astype(ml_dtypes.bfloat16)
        xt[OBS:OBS + ACTD] = ac.reshape(NBT, 128, t_steps, ACTD).transpose(
            3, 0, 2, 1).astype(ml_dtypes.bfloat16)
        xt[80] = 1.0
        in_maps.append({
            "xt": xt.reshape(128, NBT * t_steps * 128),
            "wh": wh_l, "wi": wi_l, "mlpw": mlpw_l.astype(f32),
            "mlpb": mlpb_l.astype(f32), "wout": wout_l.astype(f32),
            "boutb": boutb_l.astype(f32),
        })
    return in_maps


_RUNNER = {}


def _get_runner(t_steps=T):
    """Build the bass module once and wrap it in a cached, reusable
    shard-mapped PJRT executable (one NEFF compile per process)."""
    if t_steps in _RUNNER:
        return _RUNNER[t_steps]

    import jax
    from jax.sharding import Mesh, PartitionSpec
    from jax.experimental.shard_map import shard_map
    from concourse import bass2jax, mybir as _mb

    nc = _build(t_steps)
    bass2jax.install_neuronx_cc_hook()

    part_name = (nc.partition_id_tensor.name if nc.partition_id_tensor
                 else None)
    in_names, out_names, out_avals = [], [], []
    for alloc in nc.m.functions[0].allocations:
        if not isinstance(alloc, _mb.MemoryLocationSet):
            continue
        name = alloc.memorylocations[0].name
        if alloc.kind == "ExternalInput":
            if name != part_name:
                in_names.append(name)
        elif alloc.kind == "ExternalOutput":
            out_names.append(name)
            out_avals.append(jax.core.ShapedArray(
                tuple(alloc.tensor_shape), _mb.dt.np(alloc.dtype)))
    n_params = len(in_names)
    n_outs = len(out_avals)
    all_names = in_names + out_names
    if part_name is not None:
        all_names = all_names + [part_name]

    def _body(*args):
        operands = list(args)
        if part_name is not None:
            operands.append(bass2jax.partition_id_tensor())
        outs = bass2jax._bass_exec_p.bind(
            *operands,
            out_avals=tuple(out_avals),
            in_names=tuple(all_names),
            out_names=tuple(out_names),
            lowering_input_output_aliases=(),
            sim_require_finite=True,
            sim_require_nnan=True,
            nc=nc,
        )
        return tuple(outs)

    devices = jax.devices()[:N_CORES]
    mesh = Mesh(np.asarray(devices), ("core",))
    donate = tuple(range(n_params, n_params + n_outs))
    sharded = jax.jit(
        shard_map(_body, mesh=mesh,
                  in_specs=(PartitionSpec("core"),) * (n_params + n_outs),
                  out_specs=(PartitionSpec("core"),) * n_outs,
                  check_rep=False),
        donate_argnums=donate, keep_unused=True)

    sharded_nodon = jax.jit(
        shard_map(_body, mesh=mesh,
                  in_specs=(PartitionSpec("core"),) * (n_params + n_outs),
                  out_specs=(PartitionSpec("core"),) * n_outs,
                  check_rep=False),
        keep_unused=True)

    out_shapes = [(a.shape, a.dtype) for a in out_avals]

    def run(in_maps):
        concat_in = [
            np.concatenate([np.asarray(in_maps[c][nm]) for c in
                            range(N_CORES)], axis=0)
            for nm in in_names
        ]
        zeros = [np.zeros((N_CORES * s[0],) + tuple(s[1:]), dt)
                 for s, dt in out_shapes]
        outs = sharded(*concat_in, *zeros)
        return {nm: np.asarray(outs[i]) for i, nm in enumerate(out_names)}

    run.in_names = in_names
    run.mesh = mesh
    run.nodon = sharded_nodon
    run.out_shapes = out_shapes

    chain_cache = {}

    def make_chain(n_reps):
        """K sequential kernel executions inside one jit call, serialized
        via a zero-weight data dependency (pred -> boutb) so XLA cannot
        collapse them.  Device work scales with K while per-call dispatch
        overhead stays constant, so wall-clock differences between two K
        values isolate true device execution time."""
        if n_reps in chain_cache:
            return chain_cache[n_reps]
        import jax.numpy as jnp
        ib = in_names.index("boutb")
        ip = out_names.index("pred")

        def _chain(*args):
            ins = list(args[:n_params])
            zouts = list(args[n_params:])
            outs = None
            for _ in range(n_reps):
                outs = _body(*ins, *zouts)
                ins = list(ins)
                ins[ib] = ins[ib] + 0.0 * jnp.reshape(
                    outs[ip].astype(ins[ib].dtype), ins[ib].shape)
            return tuple(outs)

        f = jax.jit(
            shard_map(_chain, mesh=mesh,
                      in_specs=(PartitionSpec("core"),) * (n_params + n_outs),
                      out_specs=(PartitionSpec("core"),) * n_outs,
                      check_rep=False),
            keep_unused=True)
        chain_cache[n_reps] = f
        return f

    run.make_chain = make_chain
    _RUNNER[t_steps] = run
    return run


def _stage_inputs(in_maps, t_steps=T):
    """device_put concatenated inputs + zero outs once, for repeat timing."""
    import jax
    from jax.sharding import NamedSharding, PartitionSpec
    run = _get_runner(t_steps)
    sh = NamedSharding(run.mesh, PartitionSpec("core"))
    concat_in = [
        np.concatenate([np.asarray(in_maps[c][nm]) for c in range(N_CORES)],
                       axis=0)
        for nm in run.in_names
    ]
    zeros = [np.zeros((N_CORES * s[0],) + tuple(s[1:]), dt)
             for s, dt in run.out_shapes]
    return [jax.device_put(a, sh) for a in concat_in + zeros], run


def _run_staged(staged):
    arrs, run = staged
    return run.nodon(*arrs)


def kernel(trajectory, actions, Wi, Wh, bh, mlp_W, mlp_b, Wout, bout):
    run = _get_runner(T)
    in_maps = _prep_inputs(trajectory, actions, Wi, Wh, bh, mlp_W, mlp_b,
                           Wout, bout, T)
    pred = run(in_maps)["pred"]          # [8*512, 64] already batch-ordered
    return pred.astype(np.float32)
